# revision 34
# baseline (speedup 1.0000x reference)
"""Trainium2 Bass kernel for nn_Attention3D (B=4, C=256, D=H=W=16).

y = x + wp @ softmax_j((wq@x+bq)^T (wk@x+bk) / sqrt(C)) applied to (wv@x+bv), + bp

Sharding: 8 cores = (batch b, query-half). Each core owns one batch's full
K/V (N=4096 keys) and half the queries (NI=2048). Key order is permuted per
core so "my" queries are always columns 0:2048 — softmax/attention are
invariant to key permutation, so every core runs the identical program.

v3 design (PE matmul-count-minimal; every 512-wide MM costs ~215ns on HW):
  - q-projection ELIMINATED: scores s[i,j] = x_i^T (wq^T wk) x_j + (wk^T
    bq)^T x_j.  k'' = M x with M = wq^T wk (one fp8 DR matmul per tile);
    the bq term is a per-KEY scalar beta_j fed through the exp's
    per-partition bias operand (keys are the partition dim of scores).
  - bk dropped (constant over keys -> cancels in softmax); bv folded into
    the residual input (softmax rows sum to 1): xq16 = x + bp + wp@bv.
  - v+out projection fused (wvp = wp@wv) and split by SVD into pass A
    (components 0..127) and pass B (128..254, sigma_255 dropped ~1e-4):
    stationaries vA'(128 cols) and vB'(127 cols + a constant-1 column), so
    the softmax denominator S rides the attention accumulation on PSUM
    partition 127 — the 64 ones-matmuls of v2 vanish.  beta is produced by
    a 256th column (wk^T bq) of the same fused projection.
  - Normalized pA/pB (bf16) are then projected back by U-side stationaries
    (8 small bf16 MMs per chunk); a 2-MM fp16 broadcast matmul replicates
    S across partitions for the reciprocal.
  - Attention per i-chunk of 1024 queries, software-pipelined 3 superblocks
    deep with the k/v projections interleaved as PE fill work in chunk 0.
  - PSUM: scores ring 2x[128,1024]f32 (4 banks) + accumulators aA,aB
    (4 banks) = 8 exactly; S-broadcast/y-proj tiles time-share the ring.
"""

import numpy as np
import ml_dtypes

B, C = 4, 256
P = 128
D = H = W = 16
N = D * H * W          # 4096 voxels
NI = 2048              # queries per core
NCORES = 8
IC = 1024              # i-chunk
HIC = 512              # half chunk (one PSUM bank of fp32)
NICH = NI // IC        # 2 i-chunks
NJ = N // P            # 32 key blocks
NSB = NJ // 2          # 16 key superblocks (256 keys each, fp8 DoubleRow)
ESHIFT = -4 * 0.6931471805599453  # exp bias: fold 2^-4 so e fits fp8 e4m3
SCALE = float(C) ** -0.5
# fast-exp bit trick (DVE offload): e = bitcast_f32(round(s*EA + betaX_j))
L2E = 1.4426950408889634
ECORR = -0.057                    # calibrated so mean(e_approx/e_true) ~ 1
EA = (2.0 ** 23) * L2E * SCALE
# per-key part lives in betaX; (127 + corr + ESHIFT*log2e) * 2^23 is folded in
EB0 = (127.0 + ECORR + ESHIFT * L2E) * 2.0 ** 23
EBS = SCALE * L2E * 2.0 ** 23     # betaX = beta_raw * EBS + EB0
# superblocks whose r=1 key block is exp'd on DVE instead of ScalarE; their
# attention matmuls are deferred (accumulation is commutative) so the DVE
# latency stays off the critical path
OFF_SBS = frozenset((2, 5, 8, 11))
ATTN_ORDER = [0, 1, 3, 4, 2, 6, 7, 5, 9, 10, 8, 12, 13, 11, 14, 15]

_cache = {}


def _build():
    import concourse.bacc as bacc
    import concourse.mybir as mybir
    import concourse.tile as tile

    dt = mybir.dt
    f32, f16, bf16, f8 = dt.float32, dt.float16, dt.bfloat16, dt.float8e4

    nc = bacc.Bacc("TRN2", target_bir_lowering=False, debug=False)

    # [ci, pair*N + n], channel c = pair*128 + ci (pair-interleaved for DR)
    x8_d = nc.dram_tensor("x8", [P, 2 * N], f8, kind="ExternalInput")
    mk8_d = nc.dram_tensor("mk8", [P, 2 * C], f8, kind="ExternalInput")
    wv8_d = nc.dram_tensor("wvab8", [P, 2 * C], f8, kind="ExternalInput")
    ua_d = nc.dram_tensor("uab8", [P, 2 * C], f8, kind="ExternalInput")
    xq_d = nc.dram_tensor("xq16", [C, NI], bf16, kind="ExternalInput")
    y_d = nc.dram_tensor("y", [C, NI], f32, kind="ExternalOutput")

    EXP = mybir.ActivationFunctionType.Exp
    DR = mybir.MatmulPerfMode.DoubleRow
    mult = mybir.AluOpType.mult
    add = mybir.AluOpType.add

    with tile.TileContext(nc) as tc:
        with (
            tc.tile_pool(name="consts", bufs=1) as consts,
            tc.tile_pool(name="acts", bufs=1) as acts,
            tc.tile_pool(name="e16p", bufs=8) as e16p,
            tc.tile_pool(name="sm", bufs=2) as sm,
            tc.tile_pool(name="ys", bufs=4) as ys,
            tc.tile_pool(name="ps_s", bufs=2, space="PSUM") as ps_s,
            tc.tile_pool(name="ps_a", bufs=1, space="PSUM") as ps_a,
        ):
            # ---- weights + input DMAs (mk/wvab first; x8 split across queues) ----
            w_sb = {}
            for wname, wd in (("mk8", mk8_d), ("wvab8", wv8_d)):
                t = consts.tile([P, 2, C], f8, tag=wname, name=wname)
                for pair in range(2):
                    eng = nc.sync if pair == 0 else nc.gpsimd
                    eng.dma_start(out=t[:, pair, :],
                                  in_=wd.ap()[:, pair * C:(pair + 1) * C])
                w_sb[wname] = t
            x8 = acts.tile([P, 2, N], f8, tag="x8")
            for ch in range(4):
                c0, c1 = ch * (N // 4), (ch + 1) * (N // 4)
                for pair in range(2):
                    eng = nc.sync if pair == 0 else nc.gpsimd
                    eng.dma_start(out=x8[:, pair, c0:c1],
                                  in_=x8_d.ap()[:, pair * N + c0:pair * N + c1])
            uab8 = consts.tile([P, 2, C], f8, tag="uab8")
            for pair in range(2):
                eng = nc.sync if pair == 0 else nc.gpsimd
                eng.dma_start(out=uab8[:, pair, :],
                              in_=ua_d.ap()[:, pair * C:(pair + 1) * C])
            xq16 = acts.tile([P, 2, NI], bf16, tag="xq16")
            ones16f = consts.tile([1, P], f16, tag="ones16f")
            nc.vector.memset(ones16f, 1.0)

            # ---- activations ----
            k16 = acts.tile([P, 2, N], f8, tag="k16")        # [ci, pair, j]
            # [ji, jb, col]: cols 0:128 = vA, col 128 = ones (S rides the
            # B-pass on OUT PARTITION 0), cols 129:256 = vB channels
            vT8 = acts.tile([P, NJ, C], f8, tag="vT8")
            nc.vector.memset(vT8[:, :, 128], 1.0)
            beta = acts.tile([P, NJ // 4, 4], f32, tag="beta")  # exp bias / jb
            betaX = acts.tile([P, NJ // 4, 4], f32, tag="betaX")  # bit-exp bias

            # ---- projections (fp8 DR: full K=256 contraction per matmul) ----
            def emit_kproj(jc, fast=False):
                tiles = []
                for ob in range(2):
                    ps = ps_s.tile([P, IC], f32, tag="ps", name="kps")
                    for h in range(2):
                        j0 = jc * IC + h * HIC
                        nc.tensor.matmul(
                            ps[:, h * HIC:(h + 1) * HIC],
                            w_sb["mk8"][:, :, ob * P:(ob + 1) * P],
                            x8[:, :, j0:j0 + HIC],
                            start=True, stop=True, perf_mode=DR)
                    tiles.append(ps)
                if fast:   # evacuate the first key block first: it gates exp(0)
                    for ob in range(2):
                        nc.vector.tensor_copy(k16[:, ob, 0:P], tiles[ob][:, 0:P])
                    for ob in range(2):
                        nc.vector.tensor_copy(k16[:, ob, P:IC], tiles[ob][:, P:IC])
                else:
                    for ob in range(2):
                        nc.vector.tensor_copy(
                            k16[:, ob, jc * IC:(jc + 1) * IC], tiles[ob])

            def emit_vproj(g):   # group of 4 key blocks -> vA|vB|beta columns
                psv = ps_s.tile([P, 4, C], f32, tag="ps", name="vps")
                for jj in range(4):
                    jb = 4 * g + jj
                    nc.tensor.matmul(
                        psv[:, jj, :],
                        x8[:, :, jb * P:(jb + 1) * P], w_sb["wvab8"],
                        start=True, stop=True, perf_mode=DR)
                # beta first: the exps of this group's key blocks wait on it
                nc.vector.tensor_scalar(beta[:, g, :], psv[:, :, 255],
                                        SCALE, ESHIFT, op0=mult, op1=add)
                nc.vector.tensor_scalar(betaX[:, g, :], psv[:, :, 255],
                                        EBS, EB0, op0=mult, op1=add)
                nc.vector.tensor_copy(vT8[:, 4 * g:4 * g + 4, 0:128],
                                      psv[:, :, 0:128])
                nc.vector.tensor_copy(vT8[:, 4 * g:4 * g + 4, 129:256],
                                      psv[:, :, 128:255])

            # ---- attention emitters ----
            es = {}

            i32 = mybir.dt.int32

            def scores_exp(icnk, sb):
                e16 = e16p.tile([P, 2, IC], f8, tag="e16", name="e16")
                for r in range(2):
                    jb = 2 * sb + r
                    sps = ps_s.tile([P, IC], f32, tag="ps", name="sps")
                    for h in range(2):
                        i0 = icnk * IC + h * HIC
                        nc.tensor.matmul(
                            sps[:, h * HIC:(h + 1) * HIC],
                            k16[:, :, jb * P:(jb + 1) * P],
                            x8[:, :, i0:i0 + HIC],
                            start=True, stop=True, perf_mode=DR)
                    if r == 1 and sb in OFF_SBS:
                        # fast-exp on DVE+GpSimd to unload ScalarE
                        it = sm.tile([P, IC], i32, tag="i32", name="it", bufs=3)
                        nc.vector.tensor_scalar(
                            it, sps, EA, betaX[:, jb // 4, jb % 4:jb % 4 + 1],
                            op0=mult, op1=add)
                        nc.vector.tensor_copy(e16[:, r, :],
                                              it.bitcast(dt.float32))
                    else:
                        nc.scalar.activation(
                            e16[:, r, :], sps, EXP, scale=SCALE,
                            bias=beta[:, jb // 4, jb % 4:jb % 4 + 1])
                es[icnk, sb] = e16

            def attn(icnk, sb, a_ps):
                e16 = es.pop((icnk, sb))
                first, last = (sb == 0), (sb == NSB - 1)
                for pi in range(2):     # pass A (chans 0:128) / B (S + 128:255)
                    for h in range(2):
                        nc.tensor.matmul(
                            a_ps[pi][:, h * HIC:(h + 1) * HIC],
                            vT8[:, 2 * sb:2 * sb + 2, pi * P:(pi + 1) * P],
                            e16[:, :, h * HIC:(h + 1) * HIC],
                            start=first, stop=last, perf_mode=DR)

            # ---- schedule ----
            emit_kproj(0, fast=True)
            emit_vproj(0)
            emit_vproj(1)
            fills = [lambda: emit_vproj(2), lambda: emit_kproj(1),
                     lambda: emit_vproj(3), lambda: emit_vproj(4),
                     lambda: emit_kproj(2), lambda: emit_vproj(5),
                     lambda: emit_vproj(6), lambda: emit_kproj(3),
                     lambda: emit_vproj(7)]
            DEPTH = 3

            for icnk in range(NICH):
                isl = slice(icnk * IC, (icnk + 1) * IC)
                for ob in range(2):   # residual arrives late; keep DMA off the
                    nc.gpsimd.dma_start(   # critical input window
                        out=xq16[:, ob, isl],
                        in_=xq_d.ap()[ob * P:(ob + 1) * P, isl])
                a_ps = [ps_a.tile([P, IC], f32, tag=f"a{pi}", name=f"aps{pi}")
                        for pi in range(2)]
                for sb in range(NSB):
                    if (icnk, sb) not in es:   # ic1's first DEPTH pre-rolled
                        scores_exp(icnk, sb)
                    if icnk == 0 and fills:
                        fills.pop(0)()
                    # pre-roll the next chunk's first scores INSIDE this
                    # chunk so ScalarE stays busy through the epilogue
                    if icnk + 1 < NICH and sb >= NSB - DEPTH:
                        scores_exp(icnk + 1, sb - (NSB - DEPTH))
                    if sb >= DEPTH:
                        attn(icnk, ATTN_ORDER[sb - DEPTH], a_ps)
                for j in range(NSB - DEPTH, NSB):
                    attn(icnk, ATTN_ORDER[j], a_ps)

                # ---- epilogue: S-broadcast, R, normalize, project back ----
                # (half-chunk pipelined on the last chunk to shorten the tail)
                nh = 2 if icnk == NICH - 1 else 1
                S16 = sm.tile([1, IC], f16, tag="S16")
                sb_ps = ps_s.tile([P, IC], f32, tag="ps", name="sbps")
                R = sm.tile([P, IC], f32, tag="R")
                p8 = ys.tile([P, 2, IC], f8, tag="p8", name="p8")
                yps, yts = {}, {}
                for hh in range(nh):
                    hw = IC // nh
                    hsl = slice(hh * hw, (hh + 1) * hw)
                    hrange = range(hh * (2 // nh), (hh + 1) * (2 // nh))
                    nc.vector.tensor_copy(S16[:, hsl], a_ps[1][0:1, hsl])
                    for h in hrange:
                        nc.tensor.matmul(sb_ps[:, h * HIC:(h + 1) * HIC],
                                         ones16f, S16[:, h * HIC:(h + 1) * HIC],
                                         start=True, stop=True)
                    nc.vector.reciprocal_approx_fast(out=R[:, hsl],
                                                     in_=sb_ps[:, hsl])
                    for pi in range(2):
                        nc.vector.tensor_mul(p8[:, pi, hsl], a_ps[pi][:, hsl],
                                             R[:, hsl])
                    for cb in range(2):
                        if hh == 0:
                            yps[cb] = ps_s.tile([P, IC], f32, tag="ps",
                                                name="yps")
                            yts[cb] = ys.tile([P, IC], f32, tag="yt",
                                              name="yt")
                        for h in hrange:
                            # fp8 DR: contracts pass A and B in one matmul
                            nc.tensor.matmul(
                                yps[cb][:, h * HIC:(h + 1) * HIC],
                                uab8[:, :, cb * P:(cb + 1) * P],
                                p8[:, :, h * HIC:(h + 1) * HIC],
                                start=True, stop=True, perf_mode=DR)
                        # xq16 holds x + bp + wp@bv (pre-added on host)
                        i0 = icnk * IC + hh * hw
                        nc.vector.tensor_add(
                            yts[cb][:, hsl], yps[cb][:, hsl],
                            xq16[:, cb, i0:i0 + hw])
                        nc.sync.dma_start(
                            out=y_d.ap()[cb * P:(cb + 1) * P, i0:i0 + hw],
                            in_=yts[cb][:, hsl])

    nc.compile()
    return nc


def _pack_pairs(a):
    """[C, M] -> [P, 2*M]: row ci holds (pair0 cols, pair1 cols),
    channel c = pair*128 + ci."""
    Cc, M = a.shape
    return np.ascontiguousarray(
        a.reshape(2, P, M).transpose(1, 0, 2).reshape(P, 2 * M))


def _prep_inputs(x, wq, bq, wk, bk, wv, bv, wp, bp):
    f8 = ml_dtypes.float8_e4m3fn
    bf = ml_dtypes.bfloat16
    xf = np.asarray(x, np.float32).reshape(B, C, N)
    wq64 = np.asarray(wq, np.float64)
    wk64 = np.asarray(wk, np.float64)
    wv64 = np.asarray(wv, np.float64)
    wp64 = np.asarray(wp, np.float64)
    bq64 = np.asarray(bq, np.float64)

    M = wq64.T @ wk64                    # scores = x^T M x + (wk^T bq)^T x
    cvec = wk64.T @ bq64
    wvp = wp64 @ wv64                    # fused v+out projection
    U, S, Vt = np.linalg.svd(wvp)
    sA, sB = np.sqrt(S[0:128]), np.sqrt(S[128:255])
    VA = sA[:, None] * Vt[0:128]         # [128, C] pass-A v-side
    VB = sB[:, None] * Vt[128:255]       # [127, C] pass-B v-side
    UA = (U[:, 0:128] * sA).T            # [128, C] stationary (r, c)
    UB = np.zeros((P, C))
    # B-pass out partition 0 is the S ones-column; components sit on 1..127
    UB[1:128] = (U[:, 128:255] * sB).T
    wvab = np.vstack([VA, VB, cvec[None, :]])   # [256, C] fused projection
    bout = np.asarray(bp, np.float64) + wp64 @ np.asarray(bv, np.float64)
    # y-projection stationary, DR-pair layout: pair 0 = A-pass, pair 1 = B
    uab = np.stack([UA, UB], axis=1).reshape(P, 2 * C)

    shared = {
        "mk8": _pack_pairs(M.T.astype(np.float32)).astype(f8),
        "wvab8": _pack_pairs(wvab.T.astype(np.float32)).astype(f8),
        "uab8": np.ascontiguousarray(uab.astype(np.float32)).astype(f8),
    }
    in_maps = []
    for core in range(NCORES):
        b, h = core // 2, core % 2
        xs = xf[b]
        if h == 1:  # roll so this core's query half is first (key order irrelevant)
            xs = np.concatenate([xs[:, NI:], xs[:, :NI]], axis=1)
        m = dict(shared)
        m["x8"] = _pack_pairs(xs).astype(f8)
        # residual with the output bias folded in: y = a*R + (x + bout)
        m["xq16"] = np.ascontiguousarray(
            xs[:, :NI] + bout[:, None].astype(np.float32)).astype(bf)
        in_maps.append(m)
    return in_maps


def _run(inputs, trace=False, **kwargs):
    from concourse.bass_utils import run_bass_kernel_spmd

    if "nc" not in _cache:
        _cache["nc"] = _build()
    nc = _cache["nc"]
    in_maps = _prep_inputs(**inputs)
    res = run_bass_kernel_spmd(
        nc, in_maps, core_ids=list(range(NCORES)), trace=trace, **kwargs
    )
    out = np.empty((B, C, N), np.float32)
    for core in range(NCORES):
        b, h = core // 2, core % 2
        out[b][:, h * NI:(h + 1) * NI] = res.results[core]["y"]
    return out.reshape(B, C, D, H, W), res


def kernel(**inputs):
    out, _ = _run(inputs)
    return out


# revision 36
# speedup vs baseline: 1.0784x; 1.0784x over previous
"""Trainium2 Bass kernel for nn_Attention3D (B=4, C=256, D=H=W=16).

y = x + wp @ softmax_j((wq@x+bq)^T (wk@x+bk) / sqrt(C)) applied to (wv@x+bv), + bp

Sharding: 8 cores = (batch b, query-half). Each core owns one batch's full
K/V (N=4096 keys) and half the queries (NI=2048). Key order is permuted per
core so "my" queries are always columns 0:2048 — softmax/attention are
invariant to key permutation, so every core runs the identical program.

v3 design (PE matmul-count-minimal; every 512-wide MM costs ~215ns on HW):
  - q-projection ELIMINATED: scores s[i,j] = x_i^T (wq^T wk) x_j + (wk^T
    bq)^T x_j.  k'' = M x with M = wq^T wk (one fp8 DR matmul per tile);
    the bq term is a per-KEY scalar beta_j fed through the exp's
    per-partition bias operand (keys are the partition dim of scores).
  - bk dropped (constant over keys -> cancels in softmax); bv folded into
    the residual input (softmax rows sum to 1): xq16 = x + bp + wp@bv.
  - v+out projection fused (wvp = wp@wv) and split by SVD into pass A
    (components 0..127) and pass B (128..254, sigma_255 dropped ~1e-4):
    stationaries vA'(128 cols) and vB'(127 cols + a constant-1 column), so
    the softmax denominator S rides the attention accumulation on PSUM
    partition 127 — the 64 ones-matmuls of v2 vanish.  beta is produced by
    a 256th column (wk^T bq) of the same fused projection.
  - Normalized pA/pB (bf16) are then projected back by U-side stationaries
    (8 small bf16 MMs per chunk); a 2-MM fp16 broadcast matmul replicates
    S across partitions for the reciprocal.
  - Attention per i-chunk of 1024 queries, software-pipelined 3 superblocks
    deep with the k/v projections interleaved as PE fill work in chunk 0.
  - PSUM: scores ring 2x[128,1024]f32 (4 banks) + accumulators aA,aB
    (4 banks) = 8 exactly; S-broadcast/y-proj tiles time-share the ring.
"""

import numpy as np
import ml_dtypes

B, C = 4, 256
P = 128
D = H = W = 16
N = D * H * W          # 4096 voxels
NI = 2048              # queries per core
NCORES = 8
IC = 1024              # i-chunk
HIC = 512              # half chunk (one PSUM bank of fp32)
NICH = NI // IC        # 2 i-chunks
NJ = N // P            # 32 key blocks
NSB = NJ // 2          # 16 key superblocks (256 keys each, fp8 DoubleRow)
ESHIFT = -4 * 0.6931471805599453  # exp bias: fold 2^-4 so e fits fp8 e4m3
SCALE = float(C) ** -0.5
# fast-exp bit trick (DVE offload): e = bitcast_f32(round(s*EA + betaX_j))
L2E = 1.4426950408889634
ECORR = -0.057                    # calibrated so mean(e_approx/e_true) ~ 1
EA = (2.0 ** 23) * L2E * SCALE
# per-key part lives in betaX; (127 + corr + ESHIFT*log2e) * 2^23 is folded in
EB0 = (127.0 + ECORR + ESHIFT * L2E) * 2.0 ** 23
EBS = SCALE * L2E * 2.0 ** 23     # betaX = beta_raw * EBS + EB0
# superblocks whose r=1 key block is exp'd on DVE instead of ScalarE; their
# attention matmuls are deferred (accumulation is commutative) so the DVE
# latency stays off the critical path
OFF_SBS = frozenset()
ATTN_ORDER = list(range(16))

_cache = {}


def _build():
    import concourse.bacc as bacc
    import concourse.mybir as mybir
    import concourse.tile as tile

    dt = mybir.dt
    f32, f16, bf16, f8 = dt.float32, dt.float16, dt.bfloat16, dt.float8e4

    nc = bacc.Bacc("TRN2", target_bir_lowering=False, debug=False)

    # [ci, pair*N + n], channel c = pair*128 + ci (pair-interleaved for DR)
    x8_d = nc.dram_tensor("x8", [P, 2 * N], f8, kind="ExternalInput")
    mk8_d = nc.dram_tensor("mk8", [P, 2 * C], f8, kind="ExternalInput")
    wv8_d = nc.dram_tensor("wvab8", [P, 2 * C], f8, kind="ExternalInput")
    ua_d = nc.dram_tensor("uab8", [P, 2 * C], f8, kind="ExternalInput")
    xq_d = nc.dram_tensor("xq16", [C, NI], bf16, kind="ExternalInput")
    y_d = nc.dram_tensor("y", [C, NI], f32, kind="ExternalOutput")

    EXP = mybir.ActivationFunctionType.Exp
    DR = mybir.MatmulPerfMode.DoubleRow
    mult = mybir.AluOpType.mult
    add = mybir.AluOpType.add

    with tile.TileContext(nc) as tc:
        with (
            tc.tile_pool(name="consts", bufs=1) as consts,
            tc.tile_pool(name="acts", bufs=1) as acts,
            tc.tile_pool(name="e16p", bufs=10) as e16p,
            tc.tile_pool(name="sm", bufs=2) as sm,
            tc.tile_pool(name="ys", bufs=4) as ys,
            tc.tile_pool(name="ps_s", bufs=2, space="PSUM") as ps_s,
            tc.tile_pool(name="ps_a", bufs=1, space="PSUM") as ps_a,
        ):
            # ---- weights + input DMAs (mk/wvab first; x8 split across queues) ----
            w_sb = {}
            for wname, wd in (("mk8", mk8_d), ("wvab8", wv8_d)):
                t = consts.tile([P, 2, C], f8, tag=wname, name=wname)
                for pair in range(2):
                    eng = nc.sync if pair == 0 else nc.gpsimd
                    eng.dma_start(out=t[:, pair, :],
                                  in_=wd.ap()[:, pair * C:(pair + 1) * C])
                w_sb[wname] = t
            x8 = acts.tile([P, 2, N], f8, tag="x8")
            for ch in range(4):
                c0, c1 = ch * (N // 4), (ch + 1) * (N // 4)
                for pair in range(2):
                    eng = nc.sync if pair == 0 else nc.gpsimd
                    eng.dma_start(out=x8[:, pair, c0:c1],
                                  in_=x8_d.ap()[:, pair * N + c0:pair * N + c1])
            uab8 = consts.tile([P, 2, C], f8, tag="uab8")
            for pair in range(2):
                eng = nc.sync if pair == 0 else nc.gpsimd
                eng.dma_start(out=uab8[:, pair, :],
                              in_=ua_d.ap()[:, pair * C:(pair + 1) * C])
            xq16 = acts.tile([P, 2, NI], bf16, tag="xq16")
            ones16f = consts.tile([1, P], f16, tag="ones16f")
            nc.vector.memset(ones16f, 1.0)

            # ---- activations ----
            k16 = acts.tile([P, 2, N], f8, tag="k16")        # [ci, pair, j]
            # [ji, jb, col]: cols 0:128 = vA, col 128 = ones (S rides the
            # B-pass on OUT PARTITION 0), cols 129:256 = vB channels
            vT8 = acts.tile([P, NJ, C], f8, tag="vT8")
            nc.vector.memset(vT8[:, :, 128], 1.0)
            beta = acts.tile([P, NJ // 4, 4], f32, tag="beta")  # exp bias / jb
            betaX = acts.tile([P, NJ // 4, 4], f32, tag="betaX")  # bit-exp bias

            # ---- projections (fp8 DR: full K=256 contraction per matmul) ----
            def emit_kproj(jc, fast=False):
                tiles = []
                for ob in range(2):
                    ps = ps_s.tile([P, IC], f32, tag="ps", name="kps")
                    for h in range(2):
                        j0 = jc * IC + h * HIC
                        nc.tensor.matmul(
                            ps[:, h * HIC:(h + 1) * HIC],
                            w_sb["mk8"][:, :, ob * P:(ob + 1) * P],
                            x8[:, :, j0:j0 + HIC],
                            start=True, stop=True, perf_mode=DR)
                    tiles.append(ps)
                if fast:   # evacuate the first key block first: it gates exp(0)
                    for ob in range(2):
                        nc.vector.tensor_copy(k16[:, ob, 0:P], tiles[ob][:, 0:P])
                    for ob in range(2):
                        nc.vector.tensor_copy(k16[:, ob, P:IC], tiles[ob][:, P:IC])
                else:
                    for ob in range(2):
                        nc.vector.tensor_copy(
                            k16[:, ob, jc * IC:(jc + 1) * IC], tiles[ob])

            def emit_vproj(g):   # group of 4 key blocks -> vA|vB|beta columns
                psv = ps_s.tile([P, 4, C], f32, tag="ps", name="vps")
                for jj in range(4):
                    jb = 4 * g + jj
                    nc.tensor.matmul(
                        psv[:, jj, :],
                        x8[:, :, jb * P:(jb + 1) * P], w_sb["wvab8"],
                        start=True, stop=True, perf_mode=DR)
                # beta first: the exps of this group's key blocks wait on it
                nc.vector.tensor_scalar(beta[:, g, :], psv[:, :, 255],
                                        SCALE, ESHIFT, op0=mult, op1=add)
                nc.vector.tensor_scalar(betaX[:, g, :], psv[:, :, 255],
                                        EBS, EB0, op0=mult, op1=add)
                nc.vector.tensor_copy(vT8[:, 4 * g:4 * g + 4, 0:128],
                                      psv[:, :, 0:128])
                nc.vector.tensor_copy(vT8[:, 4 * g:4 * g + 4, 129:256],
                                      psv[:, :, 128:255])

            # ---- attention emitters ----
            es = {}

            i32 = mybir.dt.int32

            def scores_exp(icnk, sb):
                e16 = e16p.tile([P, 2, IC], f8, tag="e16", name="e16")
                for r in range(2):
                    jb = 2 * sb + r
                    sps = ps_s.tile([P, IC], f32, tag="ps", name="sps")
                    for h in range(2):
                        i0 = icnk * IC + h * HIC
                        nc.tensor.matmul(
                            sps[:, h * HIC:(h + 1) * HIC],
                            k16[:, :, jb * P:(jb + 1) * P],
                            x8[:, :, i0:i0 + HIC],
                            start=True, stop=True, perf_mode=DR)
                    if r == 1 and sb in OFF_SBS:
                        # fast-exp on DVE+GpSimd to unload ScalarE
                        it = sm.tile([P, IC], i32, tag="i32", name="it", bufs=3)
                        nc.vector.tensor_scalar(
                            it, sps, EA, betaX[:, jb // 4, jb % 4:jb % 4 + 1],
                            op0=mult, op1=add)
                        nc.vector.tensor_copy(e16[:, r, :],
                                              it.bitcast(dt.float32))
                    else:
                        nc.scalar.activation(
                            e16[:, r, :], sps, EXP, scale=SCALE,
                            bias=beta[:, jb // 4, jb % 4:jb % 4 + 1])
                es[icnk, sb] = e16

            def attn(icnk, sb, a_ps):
                e16 = es.pop((icnk, sb))
                first, last = (sb == 0), (sb == NSB - 1)
                for pi in range(2):     # pass A (chans 0:128) / B (S + 128:255)
                    for h in range(2):
                        nc.tensor.matmul(
                            a_ps[pi][:, h * HIC:(h + 1) * HIC],
                            vT8[:, 2 * sb:2 * sb + 2, pi * P:(pi + 1) * P],
                            e16[:, :, h * HIC:(h + 1) * HIC],
                            start=first, stop=last, perf_mode=DR)

            # ---- schedule ----
            emit_kproj(0, fast=True)
            emit_vproj(0)
            emit_vproj(1)
            fills = [lambda: emit_vproj(2), lambda: emit_kproj(1),
                     lambda: emit_vproj(3), lambda: emit_vproj(4),
                     lambda: emit_kproj(2), lambda: emit_vproj(5),
                     lambda: emit_vproj(6), lambda: emit_kproj(3),
                     lambda: emit_vproj(7)]
            DEPTH = 3

            for icnk in range(NICH):
                isl = slice(icnk * IC, (icnk + 1) * IC)
                for ob in range(2):   # residual arrives late; keep DMA off the
                    nc.gpsimd.dma_start(   # critical input window
                        out=xq16[:, ob, isl],
                        in_=xq_d.ap()[ob * P:(ob + 1) * P, isl])
                a_ps = [ps_a.tile([P, IC], f32, tag=f"a{pi}", name=f"aps{pi}")
                        for pi in range(2)]
                for sb in range(NSB):
                    if (icnk, sb) not in es:   # ic1's first DEPTH pre-rolled
                        scores_exp(icnk, sb)
                    if icnk == 0 and fills:
                        fills.pop(0)()
                    # pre-roll the next chunk's first scores INSIDE this
                    # chunk so ScalarE stays busy through the epilogue
                    if icnk + 1 < NICH and sb >= NSB - DEPTH:
                        scores_exp(icnk + 1, sb - (NSB - DEPTH))
                    if sb >= DEPTH:
                        attn(icnk, ATTN_ORDER[sb - DEPTH], a_ps)
                for j in range(NSB - DEPTH, NSB):
                    attn(icnk, ATTN_ORDER[j], a_ps)

                # ---- epilogue: S-broadcast, R, normalize, project back ----
                # (half-chunk pipelined on the last chunk to shorten the tail)
                nh = 2 if icnk == NICH - 1 else 1
                S16 = sm.tile([1, IC], f16, tag="S16")
                sb_ps = ps_s.tile([P, IC], f32, tag="ps", name="sbps")
                R = sm.tile([P, IC], f32, tag="R")
                p8 = ys.tile([P, 2, IC], f8, tag="p8", name="p8")
                yps, yts = {}, {}
                for hh in range(nh):
                    hw = IC // nh
                    hsl = slice(hh * hw, (hh + 1) * hw)
                    hrange = range(hh * (2 // nh), (hh + 1) * (2 // nh))
                    nc.vector.tensor_copy(S16[:, hsl], a_ps[1][0:1, hsl])
                    for h in hrange:
                        nc.tensor.matmul(sb_ps[:, h * HIC:(h + 1) * HIC],
                                         ones16f, S16[:, h * HIC:(h + 1) * HIC],
                                         start=True, stop=True)
                    nc.vector.reciprocal_approx_fast(out=R[:, hsl],
                                                     in_=sb_ps[:, hsl])
                    for pi in range(2):
                        nc.vector.tensor_mul(p8[:, pi, hsl], a_ps[pi][:, hsl],
                                             R[:, hsl])
                    for cb in range(2):
                        if hh == 0:
                            yps[cb] = ps_s.tile([P, IC], f32, tag="ps",
                                                name="yps")
                            yts[cb] = ys.tile([P, IC], f32, tag="yt",
                                              name="yt")
                        for h in hrange:
                            # fp8 DR: contracts pass A and B in one matmul
                            nc.tensor.matmul(
                                yps[cb][:, h * HIC:(h + 1) * HIC],
                                uab8[:, :, cb * P:(cb + 1) * P],
                                p8[:, :, h * HIC:(h + 1) * HIC],
                                start=True, stop=True, perf_mode=DR)
                        # xq16 holds x + bp + wp@bv (pre-added on host)
                        i0 = icnk * IC + hh * hw
                        nc.vector.tensor_add(
                            yts[cb][:, hsl], yps[cb][:, hsl],
                            xq16[:, cb, i0:i0 + hw])
                        nc.sync.dma_start(
                            out=y_d.ap()[cb * P:(cb + 1) * P, i0:i0 + hw],
                            in_=yts[cb][:, hsl])

    nc.compile()
    return nc


def _pack_pairs(a):
    """[C, M] -> [P, 2*M]: row ci holds (pair0 cols, pair1 cols),
    channel c = pair*128 + ci."""
    Cc, M = a.shape
    return np.ascontiguousarray(
        a.reshape(2, P, M).transpose(1, 0, 2).reshape(P, 2 * M))


def _prep_inputs(x, wq, bq, wk, bk, wv, bv, wp, bp):
    f8 = ml_dtypes.float8_e4m3fn
    bf = ml_dtypes.bfloat16
    xf = np.asarray(x, np.float32).reshape(B, C, N)
    wq64 = np.asarray(wq, np.float64)
    wk64 = np.asarray(wk, np.float64)
    wv64 = np.asarray(wv, np.float64)
    wp64 = np.asarray(wp, np.float64)
    bq64 = np.asarray(bq, np.float64)

    M = wq64.T @ wk64                    # scores = x^T M x + (wk^T bq)^T x
    cvec = wk64.T @ bq64
    wvp = wp64 @ wv64                    # fused v+out projection
    U, S, Vt = np.linalg.svd(wvp)
    sA, sB = np.sqrt(S[0:128]), np.sqrt(S[128:255])
    VA = sA[:, None] * Vt[0:128]         # [128, C] pass-A v-side
    VB = sB[:, None] * Vt[128:255]       # [127, C] pass-B v-side
    UA = (U[:, 0:128] * sA).T            # [128, C] stationary (r, c)
    UB = np.zeros((P, C))
    # B-pass out partition 0 is the S ones-column; components sit on 1..127
    UB[1:128] = (U[:, 128:255] * sB).T
    wvab = np.vstack([VA, VB, cvec[None, :]])   # [256, C] fused projection
    bout = np.asarray(bp, np.float64) + wp64 @ np.asarray(bv, np.float64)
    # y-projection stationary, DR-pair layout: pair 0 = A-pass, pair 1 = B
    uab = np.stack([UA, UB], axis=1).reshape(P, 2 * C)

    shared = {
        "mk8": _pack_pairs(M.T.astype(np.float32)).astype(f8),
        "wvab8": _pack_pairs(wvab.T.astype(np.float32)).astype(f8),
        "uab8": np.ascontiguousarray(uab.astype(np.float32)).astype(f8),
    }
    in_maps = []
    for core in range(NCORES):
        b, h = core // 2, core % 2
        xs = xf[b]
        if h == 1:  # roll so this core's query half is first (key order irrelevant)
            xs = np.concatenate([xs[:, NI:], xs[:, :NI]], axis=1)
        m = dict(shared)
        m["x8"] = _pack_pairs(xs).astype(f8)
        # residual with the output bias folded in: y = a*R + (x + bout)
        m["xq16"] = np.ascontiguousarray(
            xs[:, :NI] + bout[:, None].astype(np.float32)).astype(bf)
        in_maps.append(m)
    return in_maps


def _run(inputs, trace=False, **kwargs):
    from concourse.bass_utils import run_bass_kernel_spmd

    if "nc" not in _cache:
        _cache["nc"] = _build()
    nc = _cache["nc"]
    in_maps = _prep_inputs(**inputs)
    res = run_bass_kernel_spmd(
        nc, in_maps, core_ids=list(range(NCORES)), trace=trace, **kwargs
    )
    out = np.empty((B, C, N), np.float32)
    for core in range(NCORES):
        b, h = core // 2, core % 2
        out[b][:, h * NI:(h + 1) * NI] = res.results[core]["y"]
    return out.reshape(B, C, D, H, W), res


def kernel(**inputs):
    out, _ = _run(inputs)
    return out


# revision 41
# speedup vs baseline: 1.2126x; 1.1244x over previous
"""Trainium2 Bass kernel for nn_Attention3D (B=4, C=256, D=H=W=16).

y = x + wp @ softmax_j((wq@x+bq)^T (wk@x+bk) / sqrt(C)) applied to (wv@x+bv), + bp

Sharding: 8 cores = (batch b, query-half). Each core owns one batch's full
K/V (N=4096 keys) and half the queries (2048). Key order is permuted per
core so "my" queries are always columns 0:2048 — softmax/attention are
invariant to key permutation, so every core runs the identical program.

Per-core pipeline:
  projections (bf16 matmul, fp32 PSUM): q,k channel-major; the v- and
    out-projections are FUSED into one key-major projection pvT = x^T wvp
    + bvp with host-precomputed wvp = (wp@wv)^T, bvp = wp@bv — built
    directly transposed, so no on-chip transposes are ever needed and the
    attention matmul itself emits the projected output
  attention (fp8 e4m3, DoubleRow: virtual K=256, 2 MACs/cycle):
    sT[j,i] = k^T q       exp on ScalarE (scale 1/16, output scaled 2^-4
    so e fits e4m3; the scale cancels in the softmax ratio)
    out_un = pvT^T e      S = ones^T e   (normalization deferred to the
    end; the denominator is split between TensorE — DoubleRow ones-matmul
    on even superblocks, trailing so the PE stream never waits on exp or
    the reciprocal — and VectorE, which accumulates the odd superblocks
    in SBUF, folded back in by two bf16 matmuls per chunk)
  epilogue: y = out_un * (1/S) + bp + x   in fp32, straight from PSUM
The residual input is streamed per i-chunk and the wq/wk weights are queued
ahead of the bulk x16 DMA, so the projections start as soon as data lands.
"""

import numpy as np
import ml_dtypes

B, C = 4, 256
D = H = W = 16
N = D * H * W          # 4096 voxels
P = 128                # partitions
CB = C // P            # 2 channel blocks
NI = N // 2            # 2048 queries per core
NCORES = 8
IC = 512               # i-chunk (one PSUM bank of fp32)
NIC = NI // IC         # 4 i-chunks
NJ = N // P            # 32 key blocks
NJ2 = NJ // 2          # 16 key superblocks (256 keys each, fp8 DoubleRow)
NKC = N // IC          # 8 key projection chunks
ESHIFT = -4 * 0.6931471805599453  # exp bias: fold 2^-4 so e fits fp8 e4m3

_cache = {}


def _build():
    import concourse.bacc as bacc
    import concourse.mybir as mybir
    import concourse.tile as tile

    dt = mybir.dt
    f32, bf16, f8 = dt.float32, dt.bfloat16, dt.float8e4

    nc = bacc.Bacc("TRN2", target_bir_lowering=False, debug=False)

    # [ci, pair*N + n] channel c = pair*128 + ci (pair-interleaved for DR)
    x8_d = nc.dram_tensor("x8", [P, 2 * N], f8, kind="ExternalInput")
    xq_d = nc.dram_tensor("xq32", [C, NI], f32, kind="ExternalInput")
    w_d = {
        w: nc.dram_tensor(w, [P, 2 * C], f8, kind="ExternalInput")
        for w in ("wqT", "wkT", "wvpT")
    }
    bq_d = nc.dram_tensor("bq", [C, 1], f32, kind="ExternalInput")
    bk_d = nc.dram_tensor("bk", [C, 1], f32, kind="ExternalInput")
    bv_d = nc.dram_tensor("bvp", [1, C], f32, kind="ExternalInput")
    bp_d = nc.dram_tensor("bp", [C, 1], f32, kind="ExternalInput")
    y_d = nc.dram_tensor("y", [C, NI], f32, kind="ExternalOutput")

    add = mybir.AluOpType.add
    EXP = mybir.ActivationFunctionType.Exp
    DR = mybir.MatmulPerfMode.DoubleRow

    with tile.TileContext(nc) as tc:
        with (
            tc.tile_pool(name="consts", bufs=1) as consts,
            tc.tile_pool(name="acts", bufs=1) as acts,
            tc.tile_pool(name="e16p", bufs=20) as e16p,
            tc.tile_pool(name="small", bufs=2) as small,
            tc.tile_pool(name="ys", bufs=3) as ys,
            tc.tile_pool(name="ps_s", bufs=2, space="PSUM") as ps_s,
            tc.tile_pool(name="ps_a", bufs=3, space="PSUM") as ps_a,
            tc.tile_pool(name="ps_S", bufs=1, space="PSUM") as ps_S,
        ):
            # ---- constants ----
            bias_sb = {}
            for bname, bd in (("bq", bq_d), ("bk", bk_d), ("bp", bp_d)):
                bias_sb[bname] = []
                for cb in range(CB):
                    t = consts.tile([P, 1], f32, tag=f"{bname}{cb}", name=f"{bname}{cb}")
                    nc.gpsimd.dma_start(out=t, in_=bd.ap()[cb * P:(cb + 1) * P, :])
                    bias_sb[bname].append(t)
            bv_b = consts.tile([P, C], f32, tag="bv_b")
            nc.gpsimd.dma_start(out=bv_b, in_=bv_d.ap().to_broadcast([P, C]))
            ones8_t = consts.tile([P, 2, P], f8, tag="ones8")
            nc.vector.memset(ones8_t, 1.0)
            ones16_t = consts.tile([P, P], bf16, tag="ones16")
            nc.vector.memset(ones16_t, 1.0)
            eshift_t = consts.tile([P, 1], f32, tag="eshift")
            nc.vector.memset(eshift_t, ESHIFT)

            # weight DMAs: wq/wk before the x8 bulk (needed first), wv/wp after
            w_sb = {}

            def load_w(wname):
                t = consts.tile([P, 2, C], f8, tag=wname, name=wname)
                for pair in range(2):
                    eng = nc.sync if pair == 0 else nc.gpsimd
                    eng.dma_start(out=t[:, pair, :],
                                  in_=w_d[wname].ap()[:, pair * C:(pair + 1) * C])
                w_sb[wname] = t
            # ---- input x (fp8 pair-interleaved), split DMAs so queues
            #      parallelize; query half (cols 0:2048) first ----
            x8 = acts.tile([P, 2, N], f8, tag="x8")
            load_w("wqT")
            load_w("wkT")
            for ch in range(4):
                csl = slice(ch * (N // 4), (ch + 1) * (N // 4))
                for pair in range(2):
                    eng = nc.sync if pair == 0 else nc.gpsimd
                    eng.dma_start(out=x8[:, pair, csl],
                                  in_=x8_d.ap()[:, pair * N + ch * (N // 4):
                                                pair * N + (ch + 1) * (N // 4)])
            load_w("wvpT")


            # residual input, streamed per i-chunk during phase 2
            xq32 = [acts.tile([P, NI], f32, tag=f"xq32_{cb}", name=f"xq32_{cb}")
                    for cb in range(CB)]

            # activations: fp8 pair-interleaved (channel c = pair*128 + ci)
            q16 = acts.tile([P, CB, NI], f8, tag="q16")    # [ci, pair, i]
            k16 = acts.tile([P, CB, N], f8, tag="k16")     # [ci, pair, j]
            vT16 = acts.tile([P, NJ2, 2, C], f8, tag="vT16")  # [ji, sb, pair, c]

            # ---- projections (fp8 DR: full K=256 contraction per matmul) ----
            def emit_qproj(ic):
                isl = slice(ic * IC, (ic + 1) * IC)
                for ob in range(CB):
                    ps = ps_s.tile([P, IC], f32, tag="ps_s", name="qps")
                    nc.tensor.matmul(
                        ps, w_sb["wqT"][:, :, ob * P:(ob + 1) * P],
                        x8[:, :, isl],
                        start=True, stop=True, perf_mode=DR)
                    nc.scalar.add(q16[:, ob, isl], ps, bias_sb["bq"][ob])

            def emit_kproj(jc):
                jsl = slice(jc * IC, (jc + 1) * IC)
                for ob in range(CB):
                    ps = ps_a.tile([P, IC], f32, tag="ps_a", name="kps")
                    nc.tensor.matmul(
                        ps, w_sb["wkT"][:, :, ob * P:(ob + 1) * P],
                        x8[:, :, jsl],
                        start=True, stop=True, perf_mode=DR)
                    nc.vector.tensor_scalar_add(k16[:, ob, jsl], ps, bias_sb["bk"][ob])

            def emit_vproj(j):
                ps = ps_a.tile([P, C], f32, tag="ps_a", name="vps")
                nc.tensor.matmul(
                    ps, x8[:, :, j * P:(j + 1) * P], w_sb["wvpT"],
                    start=True, stop=True, perf_mode=DR)
                nc.vector.tensor_add(vT16[:, j // 2, j % 2, :], ps, bv_b)

            for ic in range(NIC):
                emit_qproj(ic)
            for jc in range(NKC):
                emit_kproj(jc)
            for j in range(NJ):
                emit_vproj(j)

            # ---- attention, one i-chunk (512 queries) at a time ----
            for ic in range(NIC):
                isl = slice(ic * IC, (ic + 1) * IC)
                for cb in range(CB):
                    nc.gpsimd.dma_start(out=xq32[cb][:, isl],
                                        in_=xq_d.ap()[cb * P:(cb + 1) * P, isl])
                a_ps = [ps_a.tile([P, IC], f32, tag="ps_a", name=f"a_ps{cb}")
                        for cb in range(CB)]
                S_ps = ps_S.tile([P, IC], f32, tag="ps_S")
                e_pe = []
                for sb in range(NJ2):
                    s_ps = ps_s.tile([P, 2 * IC], f32, tag="ps_s")
                    for r in range(2):
                        jb = 2 * sb + r
                        nc.tensor.matmul(
                            s_ps[:, r * IC:(r + 1) * IC],
                            k16[:, :, jb * P:(jb + 1) * P], q16[:, :, isl],
                            start=True, stop=True, perf_mode=DR)
                    e16 = e16p.tile([P, 2, IC], f8, tag="e16")
                    nc.scalar.activation(e16, s_ps, EXP,
                                         scale=float(C) ** -0.5, bias=eshift_t)
                    first, last = (sb == 0), (sb == NJ2 - 1)
                    for cb in range(CB):
                        nc.tensor.matmul(
                            a_ps[cb], vT16[:, sb, :, cb * P:(cb + 1) * P], e16,
                            start=first, stop=last, perf_mode=DR)
                    # S split: odd sbs (except the last) accumulate on VectorE;
                    # the rest go through PE as trailing DoubleRow ones-matmuls
                    # so the PE stream never waits on exp or the reciprocal
                    if sb % 2 == 1 and sb < NJ2 - 1:
                        if sb == 1:
                            Sacc = small.tile([P, 2, IC], bf16, tag="Sacc")
                            nc.vector.tensor_copy(Sacc, e16)
                        else:
                            nc.vector.tensor_add(Sacc, Sacc, e16)
                    else:
                        e_pe.append(e16)
                        if len(e_pe) > 1:
                            nc.tensor.matmul(S_ps, ones8_t, e_pe.pop(0),
                                             start=(sb == 2), stop=False,
                                             perf_mode=DR)
                    if sb == NJ2 - 2:
                        # Sacc is complete (last odd sb was 13): fold it into
                        # S_ps now so the end-of-chunk chain is shorter
                        for r in range(2):
                            nc.tensor.matmul(S_ps, ones16_t, Sacc[:, r, :],
                                             start=False, stop=False)
                for n_, t in enumerate(e_pe):
                    nc.tensor.matmul(S_ps, ones8_t, t,
                                     start=False, stop=(n_ == len(e_pe) - 1),
                                     perf_mode=DR)
                R = small.tile([P, IC], f32, tag="R")
                nc.vector.reciprocal_approx_fast(out=R, in_=S_ps)
                for ob in range(CB):
                    tmp = ys.tile([P, IC], f32, tag="tmp")
                    nc.vector.tensor_mul(tmp, a_ps[ob], R)
                    yt = ys.tile([P, IC], f32, tag="yt")
                    nc.vector.scalar_tensor_tensor(
                        yt, tmp, bias_sb["bp"][ob], xq32[ob][:, isl],
                        op0=add, op1=add)
                    nc.sync.dma_start(out=y_d.ap()[ob * P:(ob + 1) * P, isl], in_=yt)

    nc.compile()
    return nc


def _pack_pairs(a):
    """[C, M] -> [P, 2*M]: row ci holds (pair0 cols, pair1 cols),
    channel c = pair*128 + ci."""
    Cc, M = a.shape
    return np.ascontiguousarray(
        a.reshape(2, P, M).transpose(1, 0, 2).reshape(P, 2 * M))


def _prep_inputs(x, wq, bq, wk, bk, wv, bv, wp, bp):
    f8 = ml_dtypes.float8_e4m3fn
    xf = np.asarray(x, np.float32).reshape(B, C, N)
    wp64 = np.asarray(wp, np.float64)
    wv64 = np.asarray(wv, np.float64)
    shared = {
        "wqT": _pack_pairs(np.asarray(wq, np.float32).T).astype(f8),
        "wkT": _pack_pairs(np.asarray(wk, np.float32).T).astype(f8),
        # out-projection folded into the v-projection: wp @ (v·p) == (wvp^T x)·p
        "wvpT": _pack_pairs(
            (wp64 @ wv64).T.astype(np.float32)).astype(f8),
        "bq": np.asarray(bq, np.float32).reshape(C, 1),
        "bk": np.asarray(bk, np.float32).reshape(C, 1),
        "bvp": (wp64 @ np.asarray(bv, np.float64)).astype(np.float32).reshape(1, C),
        "bp": np.asarray(bp, np.float32).reshape(C, 1),
    }
    in_maps = []
    for core in range(NCORES):
        b, h = core // 2, core % 2
        xs = xf[b]
        if h == 1:  # roll so this core's query half is first (key order irrelevant)
            xs = np.concatenate([xs[:, NI:], xs[:, :NI]], axis=1)
        m = dict(shared)
        m["x8"] = _pack_pairs(xs).astype(f8)
        m["xq32"] = np.ascontiguousarray(xs[:, :NI], np.float32)
        in_maps.append(m)
    return in_maps


def _run(inputs, trace=False, **kwargs):
    from concourse.bass_utils import run_bass_kernel_spmd

    if "nc" not in _cache:
        _cache["nc"] = _build()
    nc = _cache["nc"]
    in_maps = _prep_inputs(**inputs)
    res = run_bass_kernel_spmd(
        nc, in_maps, core_ids=list(range(NCORES)), trace=trace, **kwargs
    )
    out = np.empty((B, C, N), np.float32)
    for core in range(NCORES):
        b, h = core // 2, core % 2
        out[b][:, h * NI:(h + 1) * NI] = res.results[core]["y"]
    return out.reshape(B, C, D, H, W), res


def kernel(**inputs):
    out, _ = _run(inputs)
    return out



# revision 48
# speedup vs baseline: 1.4078x; 1.1610x over previous
"""Trainium2 Bass kernel for nn_Attention3D (B=4, C=256, D=H=W=16).

y = x + wp @ softmax_j((wq@x+bq)^T (wk@x+bk) / sqrt(C)) applied to (wv@x+bv), + bp

Sharding: 8 cores = (batch b, query-half). Each core owns one batch's full
K/V (N=4096 keys) and half the queries (2048). Key order is permuted per
core so "my" queries are always columns 0:2048 — softmax/attention are
invariant to key permutation, so every core runs the identical program.

Per-core pipeline:
  projections (bf16 matmul, fp32 PSUM): q,k channel-major; the v- and
    out-projections are FUSED into one key-major projection pvT = x^T wvp
    + bvp with host-precomputed wvp = (wp@wv)^T, bvp = wp@bv — built
    directly transposed, so no on-chip transposes are ever needed and the
    attention matmul itself emits the projected output
  attention (fp8 e4m3, DoubleRow: virtual K=256, 2 MACs/cycle):
    sT[j,i] = k^T q       exp on ScalarE (scale 1/16, output scaled 2^-4
    so e fits e4m3; the scale cancels in the softmax ratio)
    out_un = pvT^T e      S = ones^T e   (normalization deferred to the
    end; the denominator is split between TensorE — DoubleRow ones-matmul
    on even superblocks, trailing so the PE stream never waits on exp or
    the reciprocal — and VectorE, which accumulates the odd superblocks
    in SBUF, folded back in by two bf16 matmuls per chunk)
  epilogue: y = out_un * (1/S) + bp + x   in fp32, straight from PSUM
The residual input is streamed per i-chunk and the wq/wk weights are queued
ahead of the bulk x16 DMA, so the projections start as soon as data lands.
"""

import numpy as np
import ml_dtypes

B, C = 4, 256
D = H = W = 16
N = D * H * W          # 4096 voxels
P = 128                # partitions
CB = C // P            # 2 channel blocks
NI = N // 2            # 2048 queries per core
NCORES = 8
IC = 512               # i-chunk (one PSUM bank of fp32)
NIC = NI // IC         # 4 i-chunks
NJ = N // P            # 32 key blocks
NJ2 = NJ // 2          # 16 key superblocks (256 keys each, fp8 DoubleRow)
NKC = N // IC          # 8 key projection chunks
ESHIFT = -4 * 0.6931471805599453  # exp bias: fold 2^-4 so e fits fp8 e4m3

_cache = {}


def _build():
    import concourse.bacc as bacc
    import concourse.mybir as mybir
    import concourse.tile as tile

    dt = mybir.dt
    f32, bf16, f8 = dt.float32, dt.bfloat16, dt.float8e4

    nc = bacc.Bacc("TRN2", target_bir_lowering=False, debug=False)

    # [ci, pair*N + n] channel c = pair*128 + ci (pair-interleaved for DR)
    x8_d = nc.dram_tensor("x8", [P, 2 * N], f8, kind="ExternalInput")
    xq_d = nc.dram_tensor("xq32", [C, NI], f32, kind="ExternalInput")
    w_d = {
        w: nc.dram_tensor(w, [P, 2 * C], f8, kind="ExternalInput")
        for w in ("wqT", "wkT", "wvpT")
    }
    bq_d = nc.dram_tensor("bq", [C, 1], f32, kind="ExternalInput")
    bk_d = nc.dram_tensor("bk", [C, 1], f32, kind="ExternalInput")
    bv_d = nc.dram_tensor("bvp", [1, 2 * C], f32, kind="ExternalInput")
    bp_d = nc.dram_tensor("bp", [C, 1], f32, kind="ExternalInput")
    y_d = nc.dram_tensor("y", [C, NI], f32, kind="ExternalOutput")

    add = mybir.AluOpType.add
    EXP = mybir.ActivationFunctionType.Exp
    DR = mybir.MatmulPerfMode.DoubleRow

    with tile.TileContext(nc) as tc:
        with (
            tc.tile_pool(name="consts", bufs=1) as consts,
            tc.tile_pool(name="acts", bufs=1) as acts,
            tc.tile_pool(name="e16p", bufs=20) as e16p,
            tc.tile_pool(name="small", bufs=2) as small,
            tc.tile_pool(name="ys", bufs=3) as ys,
            tc.tile_pool(name="ps_s", bufs=2, space="PSUM") as ps_s,
            tc.tile_pool(name="ps_a", bufs=3, space="PSUM") as ps_a,
            tc.tile_pool(name="ps_S", bufs=1, space="PSUM") as ps_S,
        ):
            # ---- constants ----
            bias_sb = {}
            for bname, bd in (("bq", bq_d), ("bk", bk_d), ("bp", bp_d)):
                bias_sb[bname] = []
                for cb in range(CB):
                    t = consts.tile([P, 1], f32, tag=f"{bname}{cb}", name=f"{bname}{cb}")
                    nc.gpsimd.dma_start(out=t, in_=bd.ap()[cb * P:(cb + 1) * P, :])
                    bias_sb[bname].append(t)
            bv_b = consts.tile([P, 2, C], f32, tag="bv_b")
            nc.gpsimd.dma_start(out=bv_b, in_=bv_d.ap().to_broadcast([P, 2 * C]))
            ones8_t = consts.tile([P, 2, P], f8, tag="ones8")
            nc.vector.memset(ones8_t, 1.0)
            eshift_t = consts.tile([P, 1], f32, tag="eshift")
            nc.vector.memset(eshift_t, ESHIFT)

            # weight DMAs: wq/wk before the x8 bulk (needed first), wv/wp after
            w_sb = {}

            def load_w(wname):
                t = consts.tile([P, 2, C], f8, tag=wname, name=wname)
                for pair in range(2):
                    eng = nc.sync if pair == 0 else nc.gpsimd
                    eng.dma_start(out=t[:, pair, :],
                                  in_=w_d[wname].ap()[:, pair * C:(pair + 1) * C])
                w_sb[wname] = t
            # ---- input x (fp8, pair-interleaved), split DMAs so queues
            #      parallelize; query half (cols 0:2048) first ----
            x8 = acts.tile([P, 2, N], f8, tag="x8")
            load_w("wqT")
            load_w("wkT")
            for ch in range(4):
                for pair in range(2):
                    c0, c1 = ch * (N // 4), (ch + 1) * (N // 4)
                    eng = nc.sync if pair == 0 else nc.gpsimd
                    eng.dma_start(out=x8[:, pair, c0:c1],
                                  in_=x8_d.ap()[:, pair * N + c0:pair * N + c1])
            load_w("wvpT")


            # residual input, streamed per i-chunk during phase 2
            xq32 = [acts.tile([P, NI], f32, tag=f"xq32_{cb}", name=f"xq32_{cb}")
                    for cb in range(CB)]

            # activations: fp8 pair-interleaved (channel c = pair*128 + ci)
            q16 = acts.tile([P, CB, NI], f8, tag="q16")    # [ci, pair, i]
            k16 = acts.tile([P, CB, N], f8, tag="k16")     # [ci, pair, j]
            vT16 = acts.tile([P, NJ2, 2, C], f8, tag="vT16")  # [ji, sb, pair, c]

            # ---- projections (fp8 DR: full K=256 contraction per matmul) ----
            def emit_qproj(ic):
                isl = slice(ic * IC, (ic + 1) * IC)
                for ob in range(CB):
                    ps = ps_s.tile([P, IC], f32, tag="ps_s", name="qps")
                    nc.tensor.matmul(
                        ps, w_sb["wqT"][:, :, ob * P:(ob + 1) * P],
                        x8[:, :, isl],
                        start=True, stop=True, perf_mode=DR)
                    nc.scalar.add(q16[:, ob, isl], ps, bias_sb["bq"][ob])

            def emit_kproj(jc):
                jsl = slice(jc * IC, (jc + 1) * IC)
                for ob in range(CB):
                    ps = ps_a.tile([P, IC], f32, tag="ps_a", name="kps")
                    nc.tensor.matmul(
                        ps, w_sb["wkT"][:, :, ob * P:(ob + 1) * P],
                        x8[:, :, jsl],
                        start=True, stop=True, perf_mode=DR)
                    nc.vector.tensor_scalar_add(k16[:, ob, jsl], ps, bias_sb["bk"][ob])

            def emit_vproj(sb):   # one superblock = 2 key blocks, one evac
                ps = ps_a.tile([P, 2, C], f32, tag="ps_a", name="vps")
                for r in range(2):
                    j = 2 * sb + r
                    nc.tensor.matmul(
                        ps[:, r, :], x8[:, :, j * P:(j + 1) * P],
                        w_sb["wvpT"],
                        start=True, stop=True, perf_mode=DR)
                nc.vector.tensor_add(vT16[:, sb, :, :], ps, bv_b)

            for ic in range(NIC):
                emit_qproj(ic)
            for jc in range(NKC):
                emit_kproj(jc)
            for sb in range(NJ2):
                emit_vproj(sb)

            # ---- attention, one i-chunk (512 queries) at a time ----
            for ic in range(NIC):
                isl = slice(ic * IC, (ic + 1) * IC)
                for cb in range(CB):
                    nc.gpsimd.dma_start(out=xq32[cb][:, isl],
                                        in_=xq_d.ap()[cb * P:(cb + 1) * P, isl])
                a_ps = [ps_a.tile([P, IC], f32, tag="ps_a", name=f"a_ps{cb}")
                        for cb in range(CB)]
                S_ps = ps_S.tile([P, IC], f32, tag="ps_S")
                e_pe = []
                for sb in range(NJ2):
                    s_ps = ps_s.tile([P, 2 * IC], f32, tag="ps_s")
                    for r in range(2):
                        jb = 2 * sb + r
                        nc.tensor.matmul(
                            s_ps[:, r * IC:(r + 1) * IC],
                            k16[:, :, jb * P:(jb + 1) * P], q16[:, :, isl],
                            start=True, stop=True, perf_mode=DR)
                    e16 = e16p.tile([P, 2, IC], f8, tag="e16")
                    nc.scalar.activation(e16, s_ps, EXP,
                                         scale=float(C) ** -0.5, bias=eshift_t)
                    first, last = (sb == 0), (sb == NJ2 - 1)
                    for cb in range(CB):
                        nc.tensor.matmul(
                            a_ps[cb], vT16[:, sb, :, cb * P:(cb + 1) * P], e16,
                            start=first, stop=last, perf_mode=DR)
                    # softmax denominator fully on TensorE (VectorE is the
                    # scarce engine once projections are fp8): trailing
                    # DoubleRow ones-matmuls, one superblock behind exp
                    e_pe.append(e16)
                    if len(e_pe) > 1:
                        nc.tensor.matmul(S_ps, ones8_t, e_pe.pop(0),
                                         start=(sb == 1), stop=False,
                                         perf_mode=DR)
                for n_, t in enumerate(e_pe):
                    nc.tensor.matmul(S_ps, ones8_t, t,
                                     start=False, stop=(n_ == len(e_pe) - 1),
                                     perf_mode=DR)
                R = small.tile([P, IC], f32, tag="R")
                nc.vector.reciprocal_approx_fast(out=R, in_=S_ps)
                for ob in range(CB):
                    tmp = ys.tile([P, IC], f32, tag="tmp")
                    nc.vector.tensor_mul(tmp, a_ps[ob], R)
                    yt = ys.tile([P, IC], f32, tag="yt")
                    nc.vector.scalar_tensor_tensor(
                        yt, tmp, bias_sb["bp"][ob], xq32[ob][:, isl],
                        op0=add, op1=add)
                    nc.sync.dma_start(out=y_d.ap()[ob * P:(ob + 1) * P, isl], in_=yt)

    nc.compile()
    return nc


def _pack_pairs(a):
    """[C, M] -> [P, 2*M]: row ci holds (pair0 cols, pair1 cols),
    channel c = pair*128 + ci."""
    Cc, M = a.shape
    return np.ascontiguousarray(
        a.reshape(2, P, M).transpose(1, 0, 2).reshape(P, 2 * M))


def _prep_inputs(x, wq, bq, wk, bk, wv, bv, wp, bp):
    f8 = ml_dtypes.float8_e4m3fn
    xf = np.asarray(x, np.float32).reshape(B, C, N)
    wp64 = np.asarray(wp, np.float64)
    wv64 = np.asarray(wv, np.float64)
    bvp = (wp64 @ np.asarray(bv, np.float64)).astype(np.float32)
    shared = {
        "wqT": _pack_pairs(np.asarray(wq, np.float32).T).astype(f8),
        "wkT": _pack_pairs(np.asarray(wk, np.float32).T).astype(f8),
        # out-projection folded into the v-projection: wp @ (v·p) == (wvp^T x)·p
        "wvpT": _pack_pairs((wp64 @ wv64).T.astype(np.float32)).astype(f8),
        "bq": np.asarray(bq, np.float32).reshape(C, 1),
        "bk": np.asarray(bk, np.float32).reshape(C, 1),
        "bvp": np.tile(bvp, 2).reshape(1, 2 * C),
        "bp": np.asarray(bp, np.float32).reshape(C, 1),
    }
    in_maps = []
    for core in range(NCORES):
        b, h = core // 2, core % 2
        xs = xf[b]
        if h == 1:  # roll so this core's query half is first (key order irrelevant)
            xs = np.concatenate([xs[:, NI:], xs[:, :NI]], axis=1)
        m = dict(shared)
        m["x8"] = _pack_pairs(xs).astype(f8)
        m["xq32"] = np.ascontiguousarray(xs[:, :NI], np.float32)
        in_maps.append(m)
    return in_maps


def _run(inputs, trace=False, **kwargs):
    from concourse.bass_utils import run_bass_kernel_spmd

    if "nc" not in _cache:
        _cache["nc"] = _build()
    nc = _cache["nc"]
    in_maps = _prep_inputs(**inputs)
    res = run_bass_kernel_spmd(
        nc, in_maps, core_ids=list(range(NCORES)), trace=trace, **kwargs
    )
    out = np.empty((B, C, N), np.float32)
    for core in range(NCORES):
        b, h = core // 2, core % 2
        out[b][:, h * NI:(h + 1) * NI] = res.results[core]["y"]
    return out.reshape(B, C, D, H, W), res


def kernel(**inputs):
    out, _ = _run(inputs)
    return out

